# revision 27
# baseline (speedup 1.0000x reference)
"""AGCA channel-attention forward, data-parallel across 8 TRN2 NeuronCores.

Reference computation (per batch element b):
    y[b,c]   = mean(x[b,c,:,:])                      # global avg pool
    y1[b,h]  = sum_c y[b,c] * W1[h,c]                # 1x1 conv == matmul
    a[b,:]   = softmax(w2 * y1[b,:])                 # over hidden dim
    z[b,k]   = y1[b,k]*a[b,k] + sum_h y1[b,h]*A2[h,k]
    zr       = relu(w3 * z)
    g[b,c]   = sigmoid(sum_h zr[b,h] * W4[c,h])
    out      = x * g[:, :, None, None]

Sharding: pure data parallel on batch (32 -> 4 per core); the tiny params
are replicated. No collectives.

Host-side folding (all inside kernel(), which receives the raw inputs):
  - W1/W4 are pre-transposed to the layouts the TensorEngine wants.
  - the 1/(H*W) of the mean and the w2 scalar fold into the softmax-branch
    copy of W1; sign(w3) folds into the value-branch copy (pushed through
    the linear ops so relu(w3*z) = |w3| * relu(sign(w3)*z)); |w3| folds
    into W4. No runtime scalars reach the device.
  - all params pack into ONE [128, 580] tensor -> one DMA, one funnel copy.

Per-core dataflow (fully pipelined per batch -- batches are independent
through the whole module): for each of the 4 local batches, one 2-block
load ([128 partitions, 2, 3136], row i = b*256 + c at block k = i//128,
partition i%128), a DVE row-sum, the tiny per-batch MLP on PE/ACT/DVE,
an in-place per-partition-scalar gate multiply, and a store. Loads issue
on the Sync HWDGE ring, stores on the Scalar HWDGE ring, so batch b's
store overlaps batch b+1's load and the HBM read+write streams
interleave at line rate. The shard stays resident in SBUF (12.85 MB).
"""

import numpy as np

import concourse.bacc as bacc
import concourse.bass as bass
import concourse.mybir as mybir
import concourse.tile as tile
from concourse.bass_utils import run_bass_kernel_spmd

# Problem shapes (hardcoded: kernel.py must be self-contained).
B, C, H, W = 32, 256, 56, 56
HIDE = 64
NCORES = 8
BL = B // NCORES  # batches per core = 4
HW = H * W  # 3136
ROWS = BL * C  # 1024 rows per core
KBLK = ROWS // 128  # 8 blocks of 128 rows
F32 = mybir.dt.float32
AX = mybir.AxisListType
AF = mybir.ActivationFunctionType

# Packed-parameter column layout: [w2*W1T | s3*W1T | A2 | |w3|*W4T | I4]
PCOLS_W1W2 = 0  # [128, 2*HIDE]
PCOLS_W1S = 2 * HIDE  # [128, 2*HIDE]
PCOLS_A2 = 4 * HIDE  # [64, HIDE]
PCOLS_W4 = 5 * HIDE  # [64, C]
PCOLS_I4 = 5 * HIDE + C  # [4, 4]
PCOLS = PCOLS_I4 + BL  # 580


def _build() -> bass.Bass:
    nc = bacc.Bacc("TRN2", target_bir_lowering=False)
    x_d = nc.dram_tensor("x", [KBLK, 128, HW], F32, kind="ExternalInput")
    params_d = nc.dram_tensor("PARAMS", [128, PCOLS], F32, kind="ExternalInput")
    out_d = nc.dram_tensor("out", [KBLK, 128, HW], F32, kind="ExternalOutput")

    with tile.TileContext(nc) as tc:
        with (
            tc.tile_pool(name="big", bufs=1) as big,
            tc.tile_pool(name="consts", bufs=1) as consts,
            tc.tile_pool(name="small", bufs=2) as small,
            tc.tile_pool(name="gpool", bufs=1) as gpool,
            tc.tile_pool(name="psm1", bufs=1, space="PSUM") as psm1,
            tc.tile_pool(name="psm2", bufs=2, space="PSUM") as psm2,
            tc.tile_pool(name="psg", bufs=2, space="PSUM") as psg,
        ):
            # ---- params: one DMA + one DVE funnel copy ----
            p_raw = consts.tile([128, PCOLS], F32)
            nc.gpsimd.dma_start(out=p_raw[:, :], in_=params_d[:, :])
            ps = consts.tile([128, PCOLS], F32)
            nc.vector.tensor_copy(out=ps[:, :], in_=p_raw[:, :])

            w1w2 = ps[:, PCOLS_W1W2:PCOLS_W1S].rearrange(
                "p (h d) -> p h d", h=2
            )  # [128, 2, HIDE]
            w1s = ps[:, PCOLS_W1S:PCOLS_A2].rearrange("p (h d) -> p h d", h=2)
            a2s = ps[:HIDE, PCOLS_A2:PCOLS_W4]  # [64, 64]
            w4ts = ps[:HIDE, PCOLS_W4:PCOLS_I4]  # [64, 256]
            i1 = ps[:1, PCOLS_I4 : PCOLS_I4 + 1]  # [1, 1] == 1.0

            xt = big.tile([128, KBLK, HW], F32)
            ysum = gpool.tile([128, BL, 2], F32)  # ysum[p, b, hf] = row sum
            gt = gpool.tile([128, BL, 2], F32)  # gt[p, b, hf] gates blk 2b+hf
            nc.vector.memset(ysum[:, :, :], 0.0)  # ACT accum-copies add into it

            # all loads issue upfront on the Sync HWDGE ring (no waits)
            for b in range(BL):
                nc.sync.dma_start(
                    out=xt[:, 2 * b : 2 * b + 2, :],
                    in_=x_d[2 * b : 2 * b + 2, :, :].rearrange("k p c -> p k c"),
                )

            for b in range(BL):
                # per-row spatial sums: block hf=0 on DVE, block hf=1 on ACT
                # (identity copy with free-dim accumulate) -- in parallel.
                nc.vector.reduce_sum(
                    out=ysum[:, b, 0:1], in_=xt[:, 2 * b, :], axis=AX.X
                )
                nc.scalar.activation(
                    out=xt[:, 2 * b + 1, :],
                    in_=xt[:, 2 * b + 1, :],
                    func=AF.Copy,
                    accum_out=ysum[:, b, 1:2],
                )

                # w2*y1 row-major (softmax branch) and sign(w3)*y1 col-major
                # (value branch) straight off the PE via swapped matmul roles.
                y1wp = psm2.tile([1, HIDE], F32, tag="y1")
                y1tp = psm1.tile([HIDE, 1], F32, tag="y1t")
                for h in range(2):
                    nc.tensor.matmul(
                        y1wp[:, :], ysum[:, b, h : h + 1], w1w2[:, h, :],
                        start=(h == 0), stop=(h == 1),
                    )
                for h in range(2):
                    nc.tensor.matmul(
                        y1tp[:, :], w1s[:, h, :], ysum[:, b, h : h + 1],
                        start=(h == 0), stop=(h == 1),
                    )

                # a = softmax(w2*y1) over hid (free dim)
                tw2 = small.tile([1, HIDE], F32, tag="tw2")
                nc.vector.tensor_copy(out=tw2[:, :], in_=y1wp[:, :])
                negm = small.tile([1, 1], F32, tag="negm")
                nc.vector.reduce_max(
                    out=negm[:, :], in_=y1wp[:, :], axis=AX.X, negate=True
                )
                y1ts = small.tile([HIDE, 1], F32, tag="y1ts")
                nc.vector.tensor_copy(out=y1ts[:, :], in_=y1tp[:, :])
                e = small.tile([1, HIDE], F32, tag="e")
                nc.scalar.activation(
                    out=e[:, :], in_=tw2[:, :], func=AF.Exp,
                    bias=negm[:, :], scale=1.0,
                )
                s = small.tile([1, 1], F32, tag="s")
                nc.vector.reduce_sum(out=s[:, :], in_=e[:, :], axis=AX.X)
                r = small.tile([1, 1], F32, tag="r")
                nc.vector.reciprocal(out=r[:, :], in_=s[:, :])
                a = small.tile([1, HIDE], F32, tag="a")
                nc.vector.tensor_scalar_mul(out=a[:, :], in0=e[:, :], scalar1=r[:, :])

                # zT' = s3*y1T * aT + A2^T @ (s3*y1T);  zr = relu(zT')
                atp = psm1.tile([HIDE, 1], F32, tag="at")
                nc.tensor.transpose(atp[:, :], a[:, :], i1)
                p3 = psm1.tile([HIDE, 1], F32, tag="p3")
                nc.tensor.matmul(p3[:, :], a2s, y1ts[:, :], start=True, stop=True)
                ats = small.tile([HIDE, 1], F32, tag="ats")
                nc.vector.tensor_copy(out=ats[:, :], in_=atp[:, :])
                p3s = small.tile([HIDE, 1], F32, tag="p3s")
                nc.vector.tensor_copy(out=p3s[:, :], in_=p3[:, :])
                zt = small.tile([HIDE, 1], F32, tag="zt")
                nc.vector.tensor_mul(out=zt[:, :], in0=y1ts[:, :], in1=ats[:, :])
                nc.vector.tensor_add(out=zt[:, :], in0=zt[:, :], in1=p3s[:, :])
                zr = small.tile([HIDE, 1], F32, tag="zr")
                nc.vector.tensor_scalar_max(out=zr[:, :], in0=zt[:, :], scalar1=0.0)

                # g = sigmoid(|w3| * W4 @ zr) per channel half
                for h in range(2):
                    gp = psg.tile([128, 1], F32, tag="g")
                    nc.tensor.matmul(
                        gp[:, :], w4ts[:, h * 128 : (h + 1) * 128], zr[:, :],
                        start=True, stop=True,
                    )
                    nc.scalar.activation(
                        out=gt[:, b, h : h + 1], in_=gp[:, :], func=AF.Sigmoid
                    )

                # in-place gate multiply: block hf=0 on DVE, hf=1 on ACT.
                nc.vector.tensor_scalar_mul(
                    out=xt[:, 2 * b, :],
                    in0=xt[:, 2 * b, :],
                    scalar1=gt[:, b, 0:1],
                )
                nc.scalar.mul(
                    out=xt[:, 2 * b + 1, :],
                    in_=xt[:, 2 * b + 1, :],
                    mul=gt[:, b, 1:2],
                )
                # stores ride SWDGE: their waits live on the otherwise-idle
                # Pool sequencer (not stalling ACT/SP), and the SDMA engines
                # round-robin SWDGE stores with the HWDGE loads, overlapping
                # the read and write streams.
                nc.gpsimd.dma_start(
                    out=out_d[2 * b : 2 * b + 2, :, :].rearrange("k p c -> p k c"),
                    in_=xt[:, 2 * b : 2 * b + 2, :],
                )

    nc.compile()
    return nc


_CACHE: dict = {}


def _get_nc() -> bass.Bass:
    if "nc" not in _CACHE:
        _CACHE["nc"] = _build()
    return _CACHE["nc"]


def _prep_params(inputs: dict) -> np.ndarray:
    W1 = np.asarray(inputs["W1"], dtype=np.float32)
    W4 = np.asarray(inputs["W4"], dtype=np.float32)
    w2 = float(np.asarray(inputs["w2"], dtype=np.float32)[0])
    w3 = float(np.asarray(inputs["w3"], dtype=np.float32)[0])
    A2 = np.asarray(inputs["A2"], dtype=np.float32)
    assert W1.shape == (HIDE, C) and W4.shape == (C, HIDE)

    # [p, h, hid] layout: W1T[h*128+p, hid] with the channel half h as the
    # middle axis so both halves sit in one contiguous column block.
    base = (W1 / HW).T.reshape(2, 128, HIDE).transpose(1, 0, 2)  # [128, 2, HIDE]
    s3 = float(np.sign(w3))

    params = np.zeros((128, PCOLS), dtype=np.float32)
    params[:, PCOLS_W1W2:PCOLS_W1S] = (w2 * base).reshape(128, 2 * HIDE)
    params[:, PCOLS_W1S:PCOLS_A2] = (s3 * base).reshape(128, 2 * HIDE)
    params[:HIDE, PCOLS_A2:PCOLS_W4] = A2
    params[:HIDE, PCOLS_W4:PCOLS_I4] = abs(w3) * W4.T
    params[:BL, PCOLS_I4:PCOLS] = np.eye(BL, dtype=np.float32)
    return params


def _run(inputs: dict, trace: bool = False):
    x = np.ascontiguousarray(np.asarray(inputs["x"], dtype=np.float32))
    assert x.shape == (B, C, H, W)
    params = _prep_params(inputs)

    in_maps = []
    for i in range(NCORES):
        shard = np.ascontiguousarray(x[i * BL : (i + 1) * BL].reshape(KBLK, 128, HW))
        in_maps.append({"x": shard, "PARAMS": params})

    res = run_bass_kernel_spmd(
        _get_nc(), in_maps, core_ids=list(range(NCORES)), trace=trace
    )
    outs = [r["out"].reshape(BL, C, H, W) for r in res.results]
    return np.concatenate(outs, axis=0), res


def kernel(**inputs) -> np.ndarray:
    out, _ = _run(inputs)
    return out


# revision 28
# speedup vs baseline: 1.0646x; 1.0646x over previous
"""AGCA channel-attention forward, data-parallel across 8 TRN2 NeuronCores.

Reference computation (per batch element b):
    y[b,c]   = mean(x[b,c,:,:])                      # global avg pool
    y1[b,h]  = sum_c y[b,c] * W1[h,c]                # 1x1 conv == matmul
    a[b,:]   = softmax(w2 * y1[b,:])                 # over hidden dim
    z[b,k]   = y1[b,k]*a[b,k] + sum_h y1[b,h]*A2[h,k]
    zr       = relu(w3 * z)
    g[b,c]   = sigmoid(sum_h zr[b,h] * W4[c,h])
    out      = x * g[:, :, None, None]

Sharding: pure data parallel on batch (32 -> 4 per core); the tiny params
are replicated. No collectives.

Host-side folding (all inside kernel(), which receives the raw inputs):
  - W1/W4 are pre-transposed to the layouts the TensorEngine wants.
  - the 1/(H*W) of the mean and s3 = sign(w3) (s3 := 1 when w3 == 0) fold
    into W1TS, pushed through the linear ops so relu(w3*z) =
    |w3| * relu(sign(w3)*z); |w3| folds into W4. The softmax pre-scale is
    recovered from the s3-scaled y1 with the single scalar w2*s3 (s3^2=1).
  - all params pack into ONE [128, 454] tensor -> one DMA, one funnel copy.
  - sigmoid is evaluated as 1/(1+exp(-v)) so the Scalar engine only ever
    uses the Exp activation table (no per-batch table reloads).

Per-core dataflow (fully pipelined per batch -- batches are independent
through the whole module): for each of the 4 local batches, one 2-block
load ([128 partitions, 2, 3136], row i = b*256 + c at block k = i//128,
partition i%128), row-sums split across DVE (reduce, block 0) and ACT
(accumulate-copy, block 1), the tiny per-batch MLP on PE/ACT/DVE, two
in-place per-partition-scalar gate multiplies on DVE, and a store. Loads
issue on the Sync HWDGE ring; stores ride SWDGE so their waits live on
the otherwise-idle Pool sequencer and the SDMA engines round-robin the
write stream with the read stream. The shard stays resident in SBUF.
"""

import numpy as np

import concourse.bacc as bacc
import concourse.bass as bass
import concourse.mybir as mybir
import concourse.tile as tile
from concourse.bass_utils import run_bass_kernel_spmd

# Problem shapes (hardcoded: kernel.py must be self-contained).
B, C, H, W = 32, 256, 56, 56
HIDE = 64
NCORES = 8
BL = B // NCORES  # batches per core = 4
HW = H * W  # 3136
ROWS = BL * C  # 1024 rows per core
KBLK = ROWS // 128  # 8 blocks of 128 rows
F32 = mybir.dt.float32
AX = mybir.AxisListType
AF = mybir.ActivationFunctionType

# Packed-parameter column layout: [s3*W1T | A2 | |w3|*W4T | 1.0 | w2*s3]
PCOLS_W1S = 0  # [128, 2*HIDE]
PCOLS_A2 = 2 * HIDE  # [64, HIDE]
PCOLS_W4 = 3 * HIDE  # [64, C]
PCOLS_ONE = 3 * HIDE + C  # [1, 1] == 1.0 (transpose identity)
PCOLS_W2S = PCOLS_ONE + 1  # [1, 1] == w2*s3
PCOLS = PCOLS_W2S + 1  # 450


def _build() -> bass.Bass:
    nc = bacc.Bacc("TRN2", target_bir_lowering=False)
    x_d = nc.dram_tensor("x", [KBLK, 128, HW], F32, kind="ExternalInput")
    params_d = nc.dram_tensor("PARAMS", [128, PCOLS], F32, kind="ExternalInput")
    out_d = nc.dram_tensor("out", [KBLK, 128, HW], F32, kind="ExternalOutput")

    with tile.TileContext(nc) as tc:
        with (
            tc.tile_pool(name="big", bufs=1) as big,
            tc.tile_pool(name="consts", bufs=1) as consts,
            tc.tile_pool(name="small", bufs=2) as small,
            tc.tile_pool(name="gpool", bufs=1) as gpool,
            tc.tile_pool(name="psm1", bufs=1, space="PSUM") as psm1,
            tc.tile_pool(name="psm2", bufs=2, space="PSUM") as psm2,
            tc.tile_pool(name="psg", bufs=2, space="PSUM") as psg,
        ):
            # ---- params: one DMA + one DVE funnel copy ----
            p_raw = consts.tile([128, PCOLS], F32)
            nc.gpsimd.dma_start(out=p_raw[:, :], in_=params_d[:, :])
            ps = consts.tile([128, PCOLS], F32)
            nc.vector.tensor_copy(out=ps[:, :], in_=p_raw[:, :])

            w1s = ps[:, PCOLS_W1S:PCOLS_A2].rearrange(
                "p (h d) -> p h d", h=2
            )  # [128, 2, HIDE]
            a2s = ps[:HIDE, PCOLS_A2:PCOLS_W4]  # [64, 64]
            w4ts = ps[:HIDE, PCOLS_W4:PCOLS_ONE]  # [64, 256]
            i1 = ps[:1, PCOLS_ONE : PCOLS_ONE + 1]  # [1, 1] == 1.0
            w2s = ps[:1, PCOLS_W2S : PCOLS_W2S + 1]  # [1, 1] == w2*s3

            xt = big.tile([128, KBLK, HW], F32)
            ysum = gpool.tile([128, BL, 2], F32)  # ysum[p, b, hf] = row sum
            gt = gpool.tile([128, BL, 2], F32)  # gt[p, b, hf] gates blk 2b+hf
            nc.vector.memset(ysum[:, :, :], 0.0)  # ACT accum-copies add into it

            # all loads issue upfront on the Sync HWDGE ring (no waits)
            for b in range(BL):
                nc.sync.dma_start(
                    out=xt[:, 2 * b : 2 * b + 2, :],
                    in_=x_d[2 * b : 2 * b + 2, :, :].rearrange("k p c -> p k c"),
                )

            for b in range(BL):
                # per-row spatial sums: block hf=0 on DVE, block hf=1 on ACT
                # (identity copy with free-dim accumulate) -- in parallel.
                nc.vector.reduce_sum(
                    out=ysum[:, b, 0:1], in_=xt[:, 2 * b, :], axis=AX.X
                )
                nc.scalar.activation(
                    out=xt[:, 2 * b + 1, :],
                    in_=xt[:, 2 * b + 1, :],
                    func=AF.Copy,
                    accum_out=ysum[:, b, 1:2],
                )

                # value-branch pooled projection: y1s = s3 * y @ W1^T, in row
                # form off the PE, then transposed to a column via identity.
                y1p = psm2.tile([1, HIDE], F32, tag="y1")
                for h in range(2):
                    nc.tensor.matmul(
                        y1p[:, :], ysum[:, b, h : h + 1], w1s[:, h, :],
                        start=(h == 0), stop=(h == 1),
                    )
                y1row = small.tile([1, HIDE], F32, tag="y1row")
                nc.vector.tensor_copy(out=y1row[:, :], in_=y1p[:, :])
                y1tp = psm1.tile([HIDE, 1], F32, tag="y1t")
                nc.tensor.transpose(y1tp[:, :], y1row[:, :], i1)
                y1ts = small.tile([HIDE, 1], F32, tag="y1ts")
                nc.vector.tensor_copy(out=y1ts[:, :], in_=y1tp[:, :])

                # a = softmax((w2*s3) * y1s) over hid (free dim)
                tw2 = small.tile([1, HIDE], F32, tag="tw2")
                nc.vector.tensor_scalar_mul(
                    out=tw2[:, :], in0=y1row[:, :], scalar1=w2s
                )
                negm = small.tile([1, 1], F32, tag="negm")
                nc.vector.reduce_max(
                    out=negm[:, :], in_=tw2[:, :], axis=AX.X, negate=True
                )
                e = small.tile([1, HIDE], F32, tag="e")
                nc.scalar.activation(
                    out=e[:, :], in_=tw2[:, :], func=AF.Exp,
                    bias=negm[:, :], scale=1.0,
                )
                s = small.tile([1, 1], F32, tag="s")
                nc.vector.reduce_sum(out=s[:, :], in_=e[:, :], axis=AX.X)
                r = small.tile([1, 1], F32, tag="r")
                nc.vector.reciprocal(out=r[:, :], in_=s[:, :])
                a = small.tile([1, HIDE], F32, tag="a")
                nc.vector.tensor_scalar_mul(out=a[:, :], in0=e[:, :], scalar1=r[:, :])

                # zT' = s3*y1T * aT + A2^T @ (s3*y1T);  zr = relu(zT')
                atp = psm1.tile([HIDE, 1], F32, tag="at")
                nc.tensor.transpose(atp[:, :], a[:, :], i1)
                p3 = psm1.tile([HIDE, 1], F32, tag="p3")
                nc.tensor.matmul(p3[:, :], a2s, y1ts[:, :], start=True, stop=True)
                ats = small.tile([HIDE, 1], F32, tag="ats")
                nc.vector.tensor_copy(out=ats[:, :], in_=atp[:, :])
                p3s = small.tile([HIDE, 1], F32, tag="p3s")
                nc.vector.tensor_copy(out=p3s[:, :], in_=p3[:, :])
                zt = small.tile([HIDE, 1], F32, tag="zt")
                nc.vector.tensor_mul(out=zt[:, :], in0=y1ts[:, :], in1=ats[:, :])
                nc.vector.tensor_add(out=zt[:, :], in0=zt[:, :], in1=p3s[:, :])
                zr = small.tile([HIDE, 1], F32, tag="zr")
                nc.vector.tensor_scalar_max(out=zr[:, :], in0=zt[:, :], scalar1=0.0)

                # g = sigmoid(v) = 1/(1 + exp(-v)), v = |w3| * W4 @ zr per
                # channel half; exp on ACT (Exp table stays loaded), the
                # add+reciprocal on DVE. Then the in-place gate multiply.
                for hf in range(2):
                    gp = psg.tile([128, 1], F32, tag="g")
                    nc.tensor.matmul(
                        gp[:, :], w4ts[:, hf * 128 : (hf + 1) * 128], zr[:, :],
                        start=True, stop=True,
                    )
                    ge = small.tile([128, 1], F32, tag="ge")
                    nc.scalar.activation(
                        out=ge[:, :], in_=gp[:, :], func=AF.Exp, scale=-1.0
                    )
                    gd = small.tile([128, 1], F32, tag="gd")
                    nc.vector.tensor_scalar_add(out=gd[:, :], in0=ge[:, :], scalar1=1.0)
                    nc.vector.reciprocal(out=gt[:, b, hf : hf + 1], in_=gd[:, :])
                    nc.vector.tensor_scalar_mul(
                        out=xt[:, 2 * b + hf, :],
                        in0=xt[:, 2 * b + hf, :],
                        scalar1=gt[:, b, hf : hf + 1],
                    )

                # stores ride SWDGE: waits live on the idle Pool sequencer,
                # and the SDMA engines round-robin writes with the loads.
                nc.gpsimd.dma_start(
                    out=out_d[2 * b : 2 * b + 2, :, :].rearrange("k p c -> p k c"),
                    in_=xt[:, 2 * b : 2 * b + 2, :],
                )

    nc.compile()
    return nc


_CACHE: dict = {}


def _get_nc() -> bass.Bass:
    if "nc" not in _CACHE:
        _CACHE["nc"] = _build()
    return _CACHE["nc"]


def _prep_params(inputs: dict) -> np.ndarray:
    W1 = np.asarray(inputs["W1"], dtype=np.float32)
    W4 = np.asarray(inputs["W4"], dtype=np.float32)
    w2 = float(np.asarray(inputs["w2"], dtype=np.float32)[0])
    w3 = float(np.asarray(inputs["w3"], dtype=np.float32)[0])
    A2 = np.asarray(inputs["A2"], dtype=np.float32)
    assert W1.shape == (HIDE, C) and W4.shape == (C, HIDE)

    # [p, h, hid] layout: W1T[h*128+p, hid] with the channel half h as the
    # middle axis so both halves sit in one contiguous column block.
    base = (W1 / HW).T.reshape(2, 128, HIDE).transpose(1, 0, 2)  # [128, 2, HIDE]
    s3 = 1.0 if w3 == 0.0 else float(np.sign(w3))

    params = np.zeros((128, PCOLS), dtype=np.float32)
    params[:, PCOLS_W1S:PCOLS_A2] = (s3 * base).reshape(128, 2 * HIDE)
    params[:HIDE, PCOLS_A2:PCOLS_W4] = A2
    params[:HIDE, PCOLS_W4:PCOLS_ONE] = abs(w3) * W4.T
    params[0, PCOLS_ONE] = 1.0
    params[0, PCOLS_W2S] = w2 * s3
    return params


def _run(inputs: dict, trace: bool = False):
    x = np.ascontiguousarray(np.asarray(inputs["x"], dtype=np.float32))
    assert x.shape == (B, C, H, W)
    params = _prep_params(inputs)

    in_maps = []
    for i in range(NCORES):
        shard = np.ascontiguousarray(x[i * BL : (i + 1) * BL].reshape(KBLK, 128, HW))
        in_maps.append({"x": shard, "PARAMS": params})

    res = run_bass_kernel_spmd(
        _get_nc(), in_maps, core_ids=list(range(NCORES)), trace=trace
    )
    outs = [r["out"].reshape(BL, C, H, W) for r in res.results]
    return np.concatenate(outs, axis=0), res


def kernel(**inputs) -> np.ndarray:
    out, _ = _run(inputs)
    return out
